# revision 1
# baseline (speedup 1.0000x reference)
"""GateTypeExpertLayer kernel for 8 Trainium2 NeuronCores (SPMD data-parallel).

v2: phase-B expert MLPs in bf16 (4x tensor rate, half the DMA), router in fp32.

Strategy (dense-all-experts, data-parallel over nodes):
  - Host: integer preprocessing only — histogram C[n, g] of incident-edge gate
    types per destination node (the scatter-mean becomes (C @ G) / max(cnt,1)),
    sharding over nodes, weight layout packing.
  - Device (per core, 12500 nodes padded to 12800 = 25 chunks x 512):
    Phase A: router logits in node-partition orientation via two matmuls per
      128-node subtile (content: xT-tile as stationary lhsT vs Wr; gate:
      CT-tile vs G augmented with a count column), then batched top-2 +
      sigmoid combine weights W[n, e] (dense, zeros off the top-2).
    Phase B: per chunk: hT_e = W1[e]^T @ xT (feature-partition), exact Gelu,
      y_e = hT^T @ W2[e] accumulated node-partition in PSUM, combine
      sum_e W[n,e] * y_e via tensor ops, LayerNorm, DMA out.
"""

import numpy as np
import sys

sys.path.insert(0, "/opt/trn_rl_repo")

N_CORES = 8
N = 100000
H = 128
NUM_EXPERTS = 8
NUM_GATE_TYPES = 20
LN_EPS = 1e-5
NSH = N // N_CORES            # 12500 real nodes per core
CHUNK = 512
NCHUNK = (NSH + CHUNK - 1) // CHUNK   # 25
NS = NCHUNK * CHUNK           # 12800 padded
P = 128
NSUB = CHUNK // P             # 4 subtiles per chunk
NG = NCHUNK * NSUB            # 100 (p-groups per core)

_PROGRAM_CACHE = {}


def _histogram(edge_index, edge_gate_type):
    dst = np.asarray(edge_index)[1].astype(np.int64)
    egt = np.asarray(edge_gate_type).astype(np.int64)
    return np.bincount(dst * NUM_GATE_TYPES + egt,
                       minlength=N * NUM_GATE_TYPES).reshape(
                           N, NUM_GATE_TYPES).astype(np.float32)


def _build_program(include_br, reps=1, use_gpsimd=False):
    import concourse.bacc as bacc
    import concourse.tile as tile
    import concourse.mybir as mybir
    import concourse.bass as bass

    f32 = mybir.dt.float32
    bf16 = mybir.dt.bfloat16
    i32 = mybir.dt.int32
    AF = mybir.ActivationFunctionType
    OP = mybir.AluOpType

    nc = bacc.Bacc("TRN2", target_bir_lowering=False, debug=False,
                   num_devices=N_CORES)

    xT = nc.dram_tensor("xT", [P, NS], f32, kind="ExternalInput").ap()
    xTb = nc.dram_tensor("xTb", [P, NS], bf16, kind="ExternalInput").ap()
    cta = nc.dram_tensor("cta", [NUM_GATE_TYPES + 1, NS], f32,
                         kind="ExternalInput").ap()
    wg = nc.dram_tensor("wg", [P, NUM_EXPERTS], f32, kind="ExternalInput").ap()
    gg = nc.dram_tensor("gg", [NUM_GATE_TYPES + 1, NUM_EXPERTS + 1], f32,
                        kind="ExternalInput").ap()
    brr = nc.dram_tensor("brr", [1, NUM_EXPERTS], f32, kind="ExternalInput").ap()
    w1s = nc.dram_tensor("w1s", [P, 2048], bf16, kind="ExternalInput").ap()
    w2s = nc.dram_tensor("w2s", [P, 2048], bf16, kind="ExternalInput").ap()
    out = nc.dram_tensor("out", [NSH, H], f32, kind="ExternalOutput").ap()

    def bc(sl, count, mid=False):
        # broadcast helper: append (or insert) a step-0 dim to a sliced AP
        ap = [list(d) for d in sl.ap]
        if mid:
            newap = [ap[0], [0, count]] + ap[1:]
        else:
            newap = ap + [[0, count]]
        return bass.AP(tensor=sl.tensor, offset=sl.offset, ap=newap)

    with tile.TileContext(nc) as tc:
        with tc.tile_pool(name="const", bufs=1) as constp, \
             tc.tile_pool(name="route", bufs=1) as routep:
            # constants resident in SBUF
            wg_sb = constp.tile([P, NUM_EXPERTS], f32)
            nc.sync.dma_start(out=wg_sb[:], in_=wg[:])
            gg_sb = constp.tile([NUM_GATE_TYPES + 1, NUM_EXPERTS + 1], f32)
            nc.sync.dma_start(out=gg_sb[:], in_=gg[:])
            br_sb = constp.tile([1, NUM_EXPERTS], f32)
            nc.sync.dma_start(out=br_sb[:], in_=brr[:])
            w1_sb = constp.tile([P, 2048], bf16)
            nc.sync.dma_start(out=w1_sb[:], in_=w1s[:])
            w2_sb = constp.tile([P, 2048], bf16)
            nc.sync.dma_start(out=w2_sb[:], in_=w2s[:])
            eps_sb = constp.tile([P, 1], f32)
            nc.vector.memset(eps_sb[:], LN_EPS)
            # per-expert tie-break bias: -e * 1e-6
            ebi = constp.tile([P, NUM_EXPERTS], i32)
            nc.gpsimd.iota(ebi[:], pattern=[[1, NUM_EXPERTS]], base=0,
                           channel_multiplier=0)
            ebf = constp.tile([P, NUM_EXPERTS], f32)
            nc.vector.tensor_copy(out=ebf[:], in_=ebi[:])
            nc.vector.tensor_scalar_mul(ebf[:], ebf[:], -1e-6)

            def _body():
                # ---------------- Phase A: routing ----------------
                La = routep.tile([P, NG, NUM_EXPERTS], f32)       # content logits
                Lb = routep.tile([P, NG, NUM_EXPERTS + 1], f32)   # seg_sum | cnt
                with tc.tile_pool(name="apool", bufs=3) as ap_pool, \
                     tc.tile_pool(name="apsum", bufs=2, space="PSUM") as apsum:
                    for c in range(NCHUNK):
                        xc = ap_pool.tile([P, CHUNK], f32, tag="xa")
                        nc.sync.dma_start(out=xc[:], in_=xT[:, c * CHUNK:(c + 1) * CHUNK])
                        cc = ap_pool.tile([NUM_GATE_TYPES + 1, CHUNK], f32, tag="ca")
                        nc.sync.dma_start(out=cc[:], in_=cta[:, c * CHUNK:(c + 1) * CHUNK])
                        pa = apsum.tile([P, NSUB, NUM_EXPERTS], f32, tag="pa")
                        pb = apsum.tile([P, NSUB, NUM_EXPERTS + 1], f32, tag="pb")
                        for s in range(NSUB):
                            st = (not include_br)
                            nc.tensor.matmul(out=pa[:, s, :],
                                             lhsT=xc[:, s * P:(s + 1) * P],
                                             rhs=wg_sb[:], start=True, stop=st)
                            if include_br:
                                nc.tensor.matmul(out=pa[:, s, :],
                                                 lhsT=cc[NUM_GATE_TYPES:NUM_GATE_TYPES + 1,
                                                         s * P:(s + 1) * P],
                                                 rhs=br_sb[:], start=False, stop=True)
                            nc.tensor.matmul(out=pb[:, s, :],
                                             lhsT=cc[:, s * P:(s + 1) * P],
                                             rhs=gg_sb[:], start=True, stop=True)
                        g0 = c * NSUB
                        nc.vector.tensor_copy(out=La[:, g0:g0 + NSUB, :], in_=pa[:])
                        nc.vector.tensor_copy(out=Lb[:, g0:g0 + NSUB, :], in_=pb[:])

                # batched routing math (free dim = NG*8 = 800)
                cnt = Lb[:, :, NUM_EXPERTS]                       # [P, NG] stride 9
                rec = routep.tile([P, NG], f32)
                nc.vector.tensor_scalar_max(rec[:], cnt, 1.0)
                nc.vector.reciprocal(rec[:], rec[:])
                L = routep.tile([P, NG, NUM_EXPERTS], f32)
                nc.vector.tensor_tensor(out=L[:], in0=Lb[:, :, 0:NUM_EXPERTS],
                                        in1=bc(rec[:], NUM_EXPERTS), op=OP.mult)
                nc.vector.tensor_tensor(out=L[:], in0=L[:], in1=La[:], op=OP.add)
                # tie-break bias (negligible magnitude, makes top-2 unique)
                nc.vector.tensor_tensor(out=L[:], in0=L[:],
                                        in1=bc(ebf[:], NG, mid=True), op=OP.add)
                m1 = routep.tile([P, NG], f32)
                nc.vector.tensor_reduce(out=m1[:], in_=L[:],
                                        axis=mybir.AxisListType.X, op=OP.max)
                eq1 = routep.tile([P, NG, NUM_EXPERTS], f32)
                nc.vector.tensor_tensor(out=eq1[:], in0=L[:],
                                        in1=bc(m1[:], NUM_EXPERTS), op=OP.is_equal)
                Lm = routep.tile([P, NG, NUM_EXPERTS], f32)
                nc.vector.tensor_scalar_mul(Lm[:], eq1[:], 1e30)
                nc.vector.tensor_tensor(out=Lm[:], in0=L[:], in1=Lm[:], op=OP.subtract)
                m2 = routep.tile([P, NG], f32)
                nc.vector.tensor_reduce(out=m2[:], in_=Lm[:],
                                        axis=mybir.AxisListType.X, op=OP.max)
                d = routep.tile([P, NG], f32)
                nc.vector.tensor_tensor(out=d[:], in0=m1[:], in1=m2[:], op=OP.subtract)
                w1v = routep.tile([P, NG], f32)
                nc.scalar.activation(out=w1v[:], in_=d[:], func=AF.Sigmoid)
                w1m = routep.tile([P, NG], f32)
                nc.vector.tensor_scalar(w1m[:], w1v[:], 1.0, None, op0=OP.subtract)
                eq2 = routep.tile([P, NG, NUM_EXPERTS], f32)
                nc.vector.tensor_tensor(out=eq2[:], in0=Lm[:],
                                        in1=bc(m2[:], NUM_EXPERTS), op=OP.is_equal)
                W = routep.tile([P, NG, NUM_EXPERTS], f32)
                nc.vector.tensor_tensor(out=W[:], in0=eq1[:],
                                        in1=bc(w1v[:], NUM_EXPERTS), op=OP.mult)
                t2w = routep.tile([P, NG, NUM_EXPERTS], f32)
                nc.vector.tensor_tensor(out=t2w[:], in0=eq2[:],
                                        in1=bc(w1m[:], NUM_EXPERTS), op=OP.mult)
                nc.vector.tensor_tensor(out=W[:], in0=W[:], in1=t2w[:], op=OP.subtract)

                # ---------------- Phase B: experts + combine + LN ----------------
                with tc.tile_pool(name="bpool", bufs=2) as bp, \
                     tc.tile_pool(name="hpsum", bufs=1, space="PSUM") as hpsum, \
                     tc.tile_pool(name="ypsum", bufs=2, space="PSUM") as ypsum, \
                     tc.tile_pool(name="cpool", bufs=3) as cp:
                    for c in range(NCHUNK):
                        xc = bp.tile([P, CHUNK], bf16, tag="xb")
                        nc.sync.dma_start(out=xc[:], in_=xTb[:, c * CHUNK:(c + 1) * CHUNK])
                        hs = bp.tile([P, NUM_EXPERTS, 2, CHUNK], bf16, tag="hs")
                        for ep in range(NUM_EXPERTS // 2):
                            hp = hpsum.tile([P, 2, 2, CHUNK], f32, tag="hp")
                            for ei in range(2):
                                e = ep * 2 + ei
                                for m in range(2):
                                    nc.tensor.matmul(
                                        out=hp[:, ei, m, :],
                                        lhsT=w1_sb[:, e * 256 + m * P: e * 256 + (m + 1) * P],
                                        rhs=xc[:], start=True, stop=True)
                            nc.scalar.activation(out=hs[:, ep * 2:ep * 2 + 2, :, :],
                                                 in_=hp[:], func=AF.Gelu)
                        yc = cp.tile([P, NSUB, H], f32, tag="yc")
                        for s in range(NSUB):
                            ph = ypsum.tile([P, 8, H], f32, tag="py")
                            for e in range(NUM_EXPERTS):
                                for m in range(2):
                                    nc.tensor.matmul(
                                        out=ph[:, e, :],
                                        lhsT=hs[:, e, m, s * P:(s + 1) * P],
                                        rhs=w2_sb[:, (2 * e + m) * P:(2 * e + m + 1) * P],
                                        start=(m == 0), stop=(m == 1))
                            g = c * NSUB + s
                            sA = cp.tile([P, 8, H], f32, tag="sA")
                            nc.vector.tensor_tensor(out=sA[:], in0=ph[:],
                                                    in1=bc(W[:, g, 0:8], H), op=OP.mult)
                            # expert-sum tree; first (largest) fold on idle GPSIMD
                            if use_gpsimd:
                                nc.gpsimd.tensor_add(out=sA[:, 0:4, :], in0=sA[:, 0:4, :],
                                                     in1=sA[:, 4:8, :])
                            else:
                                nc.vector.tensor_tensor(out=sA[:, 0:4, :], in0=sA[:, 0:4, :],
                                                        in1=sA[:, 4:8, :], op=OP.add)
                            nc.vector.tensor_tensor(out=sA[:, 0:2, :], in0=sA[:, 0:2, :],
                                                    in1=sA[:, 2:4, :], op=OP.add)
                            nc.vector.tensor_tensor(out=yc[:, s, :], in0=sA[:, 0, :],
                                                    in1=sA[:, 1, :], op=OP.add)
                        # chunk-batched LayerNorm over features (per 128-row group)
                        mu = cp.tile([P, NSUB], f32, tag="mu")
                        nc.vector.tensor_reduce(out=mu[:], in_=yc[:],
                                                axis=mybir.AxisListType.X, op=OP.add)
                        nc.vector.tensor_scalar_mul(mu[:], mu[:], 1.0 / H)
                        dv = cp.tile([P, NSUB, H], f32, tag="dv")
                        if use_gpsimd:
                            nc.gpsimd.tensor_sub(out=dv[:], in0=yc[:],
                                                 in1=bc(mu[:], H))
                        else:
                            nc.vector.tensor_tensor(out=dv[:], in0=yc[:],
                                                    in1=bc(mu[:], H), op=OP.subtract)
                        sq = cp.tile([P, NSUB, H], f32, tag="sq")
                        nc.scalar.activation(out=sq[:], in_=dv[:], func=AF.Square)
                        vr = cp.tile([P, NSUB], f32, tag="vr")
                        nc.vector.tensor_reduce(out=vr[:], in_=sq[:],
                                                axis=mybir.AxisListType.X, op=OP.add)
                        sd = cp.tile([P, NSUB], f32, tag="sd")
                        nc.scalar.activation(out=sd[:], in_=vr[:], func=AF.Sqrt,
                                             bias=eps_sb[:], scale=1.0 / H)
                        nc.vector.reciprocal(sd[:], sd[:])
                        o = cp.tile([P, NSUB, H], f32, tag="o")
                        nc.vector.tensor_tensor(out=o[:], in0=dv[:],
                                                in1=bc(sd[:], H), op=OP.mult)
                        n0 = c * CHUNK
                        rows = min(CHUNK, NSH - n0)
                        full = rows // P
                        if full > 0:
                            nc.sync.dma_start(
                                out=out[n0:n0 + full * P, :].rearrange(
                                    "(s p) f -> p s f", p=P),
                                in_=o[:, 0:full, :])
                        rem = rows - full * P
                        if rem > 0:
                            nc.sync.dma_start(
                                out=out[n0 + full * P:n0 + rows, :],
                                in_=o[:rem, full, :])

            for _rep in range(reps):
                _body()

    nc.compile()
    return nc


def _prep_inputs(x, C, gate_type_embed, Wr, br, W1, W2):
    import ml_dtypes
    bf = ml_dtypes.bfloat16
    x = np.ascontiguousarray(np.asarray(x, dtype=np.float32))
    G = np.asarray(gate_type_embed, dtype=np.float32)
    Wr = np.asarray(Wr, dtype=np.float32)
    br = np.asarray(br, dtype=np.float32)
    W1 = np.asarray(W1, dtype=np.float32)
    W2 = np.asarray(W2, dtype=np.float32)

    gg = np.zeros((NUM_GATE_TYPES + 1, NUM_EXPERTS + 1), dtype=np.float32)
    gg[0:NUM_GATE_TYPES, 0:NUM_EXPERTS] = G
    gg[NUM_GATE_TYPES, 0:NUM_EXPERTS] = 0.0   # br handled via brr input
    gg[0:NUM_GATE_TYPES, NUM_EXPERTS] = 1.0   # count column

    w1s = W1.transpose(1, 0, 2).reshape(P, 8 * 256).astype(bf)
    w2s = W2.reshape(8, 2, P, H).transpose(2, 0, 1, 3).reshape(P, 2048).astype(bf)

    in_maps = []
    for i in range(N_CORES):
        lo, hi = i * NSH, (i + 1) * NSH
        xs = x[lo:hi]
        xT = np.zeros((P, NS), dtype=np.float32)
        xT[:, :NSH] = xs.T
        cs = C[lo:hi]
        cta = np.zeros((NUM_GATE_TYPES + 1, NS), dtype=np.float32)
        cta[0:NUM_GATE_TYPES, :NSH] = cs.T
        cta[NUM_GATE_TYPES, :] = 1.0
        in_maps.append({
            "xT": np.ascontiguousarray(xT),
            "xTb": np.ascontiguousarray(xT.astype(bf)),
            "cta": np.ascontiguousarray(cta),
            "wg": np.ascontiguousarray(Wr),
            "gg": gg,
            "brr": np.ascontiguousarray(br.reshape(1, NUM_EXPERTS)),
            "w1s": np.ascontiguousarray(w1s),
            "w2s": np.ascontiguousarray(w2s),
        })
    return in_maps
def _fallback_numpy(x, edge_gate_type, edge_index, gate_type_embed, Wr, br,
                    W1, b1, W2, b2, ln_gamma, ln_beta):
    # exact reference recomputation on host (only for unexpected inputs)
    import jax
    import jax.numpy as jnp
    x = jnp.asarray(x); Wr = jnp.asarray(Wr); br = jnp.asarray(br)
    W1 = jnp.asarray(W1); b1 = jnp.asarray(b1)
    W2 = jnp.asarray(W2); b2 = jnp.asarray(b2)
    n = x.shape[0]
    content = x @ Wr + br
    dst = jnp.asarray(edge_index)[1]
    ge = jnp.asarray(gate_type_embed)[jnp.asarray(edge_gate_type)]
    seg = jax.ops.segment_sum(ge, dst, num_segments=n)
    cnt = jax.ops.segment_sum(jnp.ones((ge.shape[0],), x.dtype), dst,
                              num_segments=n)
    ngl = jnp.where(cnt[:, None] > 0, seg / jnp.maximum(cnt, 1.0)[:, None], 0.0)
    rl = content + ngl
    tkl, tki = jax.lax.top_k(rl, 2)
    tkg = jax.nn.softmax(tkl, axis=-1)
    h = jax.nn.gelu(jnp.einsum('nd,edh->neh', x, W1) + b1, approximate=False)
    eo = jnp.einsum('neh,ehd->ned', h, W2) + b2
    sel = jnp.take_along_axis(eo, tki[:, :, None], axis=1)
    o = jnp.sum(sel * tkg[:, :, None], axis=1)
    mu = jnp.mean(o, axis=-1, keepdims=True)
    var = jnp.mean(jnp.square(o - mu), axis=-1, keepdims=True)
    o = (o - mu) * jax.lax.rsqrt(var + LN_EPS) * jnp.asarray(ln_gamma) \
        + jnp.asarray(ln_beta)
    return np.asarray(o, dtype=np.float32)


def _patch_ambiguous(out, x, C, G, Wr, br, W1, b1, W2, b2, lg, lb):
    """Fix nodes whose top-2 selection is numerically ambiguous (near-ties).

    Device vs reference fp32 rounding can flip expert selection when router
    logits are within ~1e-5 of each other; recompute those few nodes exactly.
    """
    import math
    xd = x.astype(np.float64)
    cnt = C.sum(axis=1)
    gate = (C / np.maximum(cnt, 1.0)[:, None]).astype(np.float64) @ G.astype(np.float64)
    rl = xd @ Wr.astype(np.float64) + br.astype(np.float64) + gate
    srt = np.sort(rl, axis=1)
    gap23 = srt[:, -2] - srt[:, -3]
    gap12 = srt[:, -1] - srt[:, -2]
    amb = np.where(np.minimum(gap23, gap12) < 1e-3)[0]
    if len(amb) == 0:
        return out
    erf = np.frompyfunc(math.erf, 1, 1)
    for n in amb:
        order = np.argsort(-rl[n], kind="stable")
        i1, i2 = int(order[0]), int(order[1])
        l1, l2 = rl[n, i1], rl[n, i2]
        e1 = math.exp(0.0)
        e2 = math.exp(l2 - l1)
        w1 = e1 / (e1 + e2)
        w2 = e2 / (e1 + e2)
        acc = np.zeros(H, dtype=np.float64)
        for w, e in ((w1, i1), (w2, i2)):
            z = xd[n] @ W1[e].astype(np.float64) + b1[e].astype(np.float64)
            h = 0.5 * z * (1.0 + erf(z / math.sqrt(2.0)).astype(np.float64))
            acc += w * (h @ W2[e].astype(np.float64) + b2[e].astype(np.float64))
        mu = acc.mean()
        var = ((acc - mu) ** 2).mean()
        o = (acc - mu) / math.sqrt(var + LN_EPS)
        out[n] = (o * lg.astype(np.float64) + lb.astype(np.float64)).astype(np.float32)
    return out


def _build_sparse(cpes, reps=1):
    """cpes = per-expert chunk counts (capacity_e = cpes[e]*512 slots)."""
    import concourse.bacc as bacc
    import concourse.tile as tile
    import concourse.mybir as mybir
    import concourse.bass as bass

    f32 = mybir.dt.float32
    bf16 = mybir.dt.bfloat16
    i32 = mybir.dt.int32
    AF = mybir.ActivationFunctionType
    OP = mybir.AluOpType

    cpes = tuple(int(c) for c in cpes)
    S = 512 * sum(cpes)               # total slots
    echunks = [(e, j) for e in range(NUM_EXPERTS) for j in range(cpes[e])]

    nc = bacc.Bacc("TRN2", target_bir_lowering=False, debug=False,
                   num_devices=N_CORES)

    xT = nc.dram_tensor("xT", [P, NS], f32, kind="ExternalInput").ap()
    xg = nc.dram_tensor("xg", [P, S], bf16, kind="ExternalInput").ap()
    cta = nc.dram_tensor("cta", [NUM_GATE_TYPES + 1, NS], f32,
                         kind="ExternalInput").ap()
    wg = nc.dram_tensor("wg", [P, NUM_EXPERTS], f32, kind="ExternalInput").ap()
    gg = nc.dram_tensor("gg", [NUM_GATE_TYPES + 1, NUM_EXPERTS + 1], f32,
                        kind="ExternalInput").ap()
    w1s = nc.dram_tensor("w1s", [P, 2048], bf16, kind="ExternalInput").ap()
    w2s = nc.dram_tensor("w2s", [P, 2048], bf16, kind="ExternalInput").ap()
    mAd = nc.dram_tensor("mA", [P, NG, NUM_EXPERTS], f32, kind="ExternalInput").ap()
    mBd = nc.dram_tensor("mB", [P, NG, NUM_EXPERTS], f32, kind="ExternalInput").ap()
    idxAd = nc.dram_tensor("idxA", [P, NG], i32, kind="ExternalInput").ap()
    idxBd = nc.dram_tensor("idxB", [P, NG], i32, kind="ExternalInput").ap()
    out = nc.dram_tensor("out", [NSH, H], f32, kind="ExternalOutput").ap()

    def bc(sl, count, mid=False):
        ap = [list(d) for d in sl.ap]
        if mid:
            newap = [ap[0], [0, count]] + ap[1:]
        else:
            newap = ap + [[0, count]]
        return bass.AP(tensor=sl.tensor, offset=sl.offset, ap=newap)

    with tile.TileContext(nc) as tc:
        with tc.tile_pool(name="const", bufs=1) as constp, \
             tc.tile_pool(name="route", bufs=1) as routep, \
             tc.tile_pool(name="dram", bufs=1, space="DRAM") as dramp:
            ys = dramp.tile([S, H], f32)
            wg_sb = constp.tile([P, NUM_EXPERTS], f32)
            nc.sync.dma_start(out=wg_sb[:], in_=wg[:])
            gg_sb = constp.tile([NUM_GATE_TYPES + 1, NUM_EXPERTS + 1], f32)
            nc.sync.dma_start(out=gg_sb[:], in_=gg[:])
            w1_sb = constp.tile([P, 2048], bf16)
            nc.sync.dma_start(out=w1_sb[:], in_=w1s[:])
            w2_sb = constp.tile([P, 2048], bf16)
            nc.sync.dma_start(out=w2_sb[:], in_=w2s[:])
            mA_sb = constp.tile([P, NG, NUM_EXPERTS], f32)
            nc.sync.dma_start(out=mA_sb[:], in_=mAd[:])
            mB_sb = constp.tile([P, NG, NUM_EXPERTS], f32)
            nc.sync.dma_start(out=mB_sb[:], in_=mBd[:])
            idxA_sb = constp.tile([P, NG], i32)
            nc.sync.dma_start(out=idxA_sb[:], in_=idxAd[:])
            idxB_sb = constp.tile([P, NG], i32)
            nc.sync.dma_start(out=idxB_sb[:], in_=idxBd[:])
            eps_sb = constp.tile([P, 1], f32)
            nc.vector.memset(eps_sb[:], LN_EPS)
            from concourse import masks as _masks
            ident = constp.tile([P, P], f32)
            _masks.make_identity(nc, ident[:])
            ebi = constp.tile([P, NUM_EXPERTS], i32)
            nc.gpsimd.iota(ebi[:], pattern=[[1, NUM_EXPERTS]], base=0,
                           channel_multiplier=0)
            ebf = constp.tile([P, NUM_EXPERTS], f32)
            nc.vector.tensor_copy(out=ebf[:], in_=ebi[:])
            nc.vector.tensor_scalar_mul(ebf[:], ebf[:], -1e-6)

            def _body():
                # ---------------- Phase A: routing (same as dense) --------
                La = routep.tile([P, NG, NUM_EXPERTS], f32)
                Lb = routep.tile([P, NG, NUM_EXPERTS + 1], f32)
                with tc.tile_pool(name="apool", bufs=3) as ap_pool, \
                     tc.tile_pool(name="apsum", bufs=2, space="PSUM") as apsum:
                    for c in range(NCHUNK):
                        xc = ap_pool.tile([P, CHUNK], f32, tag="xa")
                        nc.sync.dma_start(out=xc[:], in_=xT[:, c * CHUNK:(c + 1) * CHUNK])
                        cc = ap_pool.tile([NUM_GATE_TYPES + 1, CHUNK], f32, tag="ca")
                        nc.sync.dma_start(out=cc[:], in_=cta[:, c * CHUNK:(c + 1) * CHUNK])
                        pa = apsum.tile([P, NSUB, NUM_EXPERTS], f32, tag="pa")
                        pb = apsum.tile([P, NSUB, NUM_EXPERTS + 1], f32, tag="pb")
                        for s in range(NSUB):
                            nc.tensor.matmul(out=pa[:, s, :],
                                             lhsT=xc[:, s * P:(s + 1) * P],
                                             rhs=wg_sb[:], start=True, stop=True)
                            nc.tensor.matmul(out=pb[:, s, :],
                                             lhsT=cc[:, s * P:(s + 1) * P],
                                             rhs=gg_sb[:], start=True, stop=True)
                        g0 = c * NSUB
                        nc.vector.tensor_copy(out=La[:, g0:g0 + NSUB, :], in_=pa[:])
                        nc.vector.tensor_copy(out=Lb[:, g0:g0 + NSUB, :], in_=pb[:])

                cnt = Lb[:, :, NUM_EXPERTS]
                rec = routep.tile([P, NG], f32)
                nc.vector.tensor_scalar_max(rec[:], cnt, 1.0)
                nc.vector.reciprocal(rec[:], rec[:])
                L = routep.tile([P, NG, NUM_EXPERTS], f32)
                nc.vector.tensor_tensor(out=L[:], in0=Lb[:, :, 0:NUM_EXPERTS],
                                        in1=bc(rec[:], NUM_EXPERTS), op=OP.mult)
                nc.vector.tensor_tensor(out=L[:], in0=L[:], in1=La[:], op=OP.add)
                nc.vector.tensor_tensor(out=L[:], in0=L[:],
                                        in1=bc(ebf[:], NG, mid=True), op=OP.add)
                m1 = routep.tile([P, NG], f32)
                nc.vector.tensor_reduce(out=m1[:], in_=L[:],
                                        axis=mybir.AxisListType.X, op=OP.max)
                eq1 = routep.tile([P, NG, NUM_EXPERTS], f32)
                nc.vector.tensor_tensor(out=eq1[:], in0=L[:],
                                        in1=bc(m1[:], NUM_EXPERTS), op=OP.is_equal)
                Lm = routep.tile([P, NG, NUM_EXPERTS], f32)
                nc.vector.tensor_scalar_mul(Lm[:], eq1[:], 1e30)
                nc.vector.tensor_tensor(out=Lm[:], in0=L[:], in1=Lm[:], op=OP.subtract)
                m2 = routep.tile([P, NG], f32)
                nc.vector.tensor_reduce(out=m2[:], in_=Lm[:],
                                        axis=mybir.AxisListType.X, op=OP.max)
                d = routep.tile([P, NG], f32)
                nc.vector.tensor_tensor(out=d[:], in0=m1[:], in1=m2[:], op=OP.subtract)
                w1v = routep.tile([P, NG], f32)
                nc.scalar.activation(out=w1v[:], in_=d[:], func=AF.Sigmoid)
                w1m = routep.tile([P, NG], f32)
                nc.vector.tensor_scalar(w1m[:], w1v[:], 1.0, None, op0=OP.subtract)
                eq2 = routep.tile([P, NG, NUM_EXPERTS], f32)
                nc.vector.tensor_tensor(out=eq2[:], in0=Lm[:],
                                        in1=bc(m2[:], NUM_EXPERTS), op=OP.is_equal)
                W = routep.tile([P, NG, NUM_EXPERTS], f32)
                nc.vector.tensor_tensor(out=W[:], in0=eq1[:],
                                        in1=bc(w1v[:], NUM_EXPERTS), op=OP.mult)
                t2w = routep.tile([P, NG, NUM_EXPERTS], f32)
                nc.vector.tensor_tensor(out=t2w[:], in0=eq2[:],
                                        in1=bc(w1m[:], NUM_EXPERTS), op=OP.mult)
                nc.vector.tensor_tensor(out=W[:], in0=W[:], in1=t2w[:], op=OP.subtract)
                # per-rank gate of the host-planned expert
                wA = routep.tile([P, NG], f32)
                wB = routep.tile([P, NG], f32)
                tsel = routep.tile([P, NG, NUM_EXPERTS], f32)
                nc.vector.tensor_tensor(out=tsel[:], in0=W[:], in1=mA_sb[:],
                                        op=OP.mult)
                nc.vector.tensor_reduce(out=wA[:], in_=tsel[:],
                                        axis=mybir.AxisListType.X, op=OP.add)
                nc.vector.tensor_tensor(out=tsel[:], in0=W[:], in1=mB_sb[:],
                                        op=OP.mult)
                nc.vector.tensor_reduce(out=wB[:], in_=tsel[:],
                                        axis=mybir.AxisListType.X, op=OP.add)

                # ---------------- Phase B: sparse expert MLP --------------
                with tc.tile_pool(name="bpool", bufs=3) as bp, \
                     tc.tile_pool(name="hpsum", bufs=2, space="PSUM") as hpsum, \
                     tc.tile_pool(name="ypsum", bufs=2, space="PSUM") as ypsum:
                    for k, (e, _j) in enumerate(echunks):
                        xk = bp.tile([P, CHUNK], bf16, tag="xk")
                        nc.sync.dma_start(out=xk[:], in_=xg[:, k * CHUNK:(k + 1) * CHUNK])
                        hp = hpsum.tile([P, 2, CHUNK], f32, tag="hp")
                        for m in range(2):
                            nc.tensor.matmul(
                                out=hp[:, m, :],
                                lhsT=w1_sb[:, e * 256 + m * P: e * 256 + (m + 1) * P],
                                rhs=xk[:], start=True, stop=True)
                        hsb = bp.tile([P, 2, CHUNK], bf16, tag="hsb")
                        nc.scalar.activation(out=hsb[:], in_=hp[:], func=AF.Gelu)
                        yT = ypsum.tile([P, CHUNK], f32, tag="pyT")
                        for m in range(2):
                            nc.tensor.matmul(
                                out=yT[:],
                                lhsT=w2_sb[:, (2 * e + m) * P:(2 * e + m + 1) * P],
                                rhs=hsb[:, m, :],
                                start=(m == 0), stop=(m == 1))
                        yTs = bp.tile([P, CHUNK], f32, tag="yTs")
                        nc.vector.tensor_copy(out=yTs[:], in_=yT[:])
                        yst = bp.tile([P, NSUB, H], f32, tag="yst")
                        for s in range(NSUB):
                            tp = ypsum.tile([P, H], f32, tag="tp")
                            nc.tensor.transpose(out=tp[:],
                                                in_=yTs[:, s * P:(s + 1) * P],
                                                identity=ident[:])
                            nc.vector.tensor_copy(out=yst[:, s, :], in_=tp[:])
                        nc.sync.dma_start(
                            out=ys[k * CHUNK:(k + 1) * CHUNK, :].rearrange(
                                "(s p) f -> p s f", p=P),
                            in_=yst[:])

                # ---------------- Phase C: gather + combine + LN ----------
                with tc.tile_pool(name="cpool", bufs=3) as cp:
                    for c in range(NCHUNK):
                        yc = cp.tile([P, NSUB, H], f32, tag="yc")
                        for s in range(NSUB):
                            g = c * NSUB + s
                            ya = cp.tile([P, H], f32, tag="ya")
                            nc.gpsimd.indirect_dma_start(
                                out=ya[:], out_offset=None, in_=ys[:],
                                in_offset=bass.IndirectOffsetOnAxis(
                                    ap=idxA_sb[:, g:g + 1], axis=0))
                            yb = cp.tile([P, H], f32, tag="yb")
                            nc.gpsimd.indirect_dma_start(
                                out=yb[:], out_offset=None, in_=ys[:],
                                in_offset=bass.IndirectOffsetOnAxis(
                                    ap=idxB_sb[:, g:g + 1], axis=0))
                            ysc = cp.tile([P, 2, H], f32, tag="ysc")
                            nc.vector.tensor_scalar(ysc[:, 0, :], ya[:],
                                                    wA[:, g:g + 1], None,
                                                    op0=OP.mult)
                            nc.vector.tensor_scalar(ysc[:, 1, :], yb[:],
                                                    wB[:, g:g + 1], None,
                                                    op0=OP.mult)
                            nc.vector.tensor_tensor(out=yc[:, s, :],
                                                    in0=ysc[:, 0, :],
                                                    in1=ysc[:, 1, :], op=OP.add)
                        mu = cp.tile([P, NSUB], f32, tag="mu")
                        nc.vector.tensor_reduce(out=mu[:], in_=yc[:],
                                                axis=mybir.AxisListType.X, op=OP.add)
                        nc.vector.tensor_scalar_mul(mu[:], mu[:], 1.0 / H)
                        dv = cp.tile([P, NSUB, H], f32, tag="dv")
                        nc.vector.tensor_tensor(out=dv[:], in0=yc[:],
                                                in1=bc(mu[:], H), op=OP.subtract)
                        sq = cp.tile([P, NSUB, H], f32, tag="sq")
                        nc.scalar.activation(out=sq[:], in_=dv[:], func=AF.Square)
                        vr = cp.tile([P, NSUB], f32, tag="vr")
                        nc.vector.tensor_reduce(out=vr[:], in_=sq[:],
                                                axis=mybir.AxisListType.X, op=OP.add)
                        sd = cp.tile([P, NSUB], f32, tag="sd")
                        nc.scalar.activation(out=sd[:], in_=vr[:], func=AF.Sqrt,
                                             bias=eps_sb[:], scale=1.0 / H)
                        nc.vector.reciprocal(sd[:], sd[:])
                        o = cp.tile([P, NSUB, H], f32, tag="o")
                        nc.vector.tensor_tensor(out=o[:], in0=dv[:],
                                                in1=bc(sd[:], H), op=OP.mult)
                        n0 = c * CHUNK
                        rows = min(CHUNK, NSH - n0)
                        full = rows // P
                        if full > 0:
                            nc.sync.dma_start(
                                out=out[n0:n0 + full * P, :].rearrange(
                                    "(s p) f -> p s f", p=P),
                                in_=o[:, 0:full, :])
                        rem = rows - full * P
                        if rem > 0:
                            nc.sync.dma_start(
                                out=out[n0 + full * P:n0 + rows, :],
                                in_=o[:rem, full, :])

            for _rep in range(reps):
                _body()

    nc.compile()
    return nc


def _plan(x, C, gate_type_embed, Wr, br):
    """Replicate device routing in fp32; build per-core slot plan."""
    x = np.asarray(x, dtype=np.float32)
    G = np.asarray(gate_type_embed, dtype=np.float32)
    cntf = C.sum(axis=1, dtype=np.float32)
    gate = (C / np.maximum(cntf, 1.0)[:, None]).astype(np.float32) @ G
    logits = (x @ np.asarray(Wr, np.float32)
              + np.asarray(br, np.float32)[None, :] + gate)
    logits = logits + (-1e-6) * np.arange(NUM_EXPERTS, dtype=np.float32)[None, :]
    order = np.argsort(-logits, axis=1, kind="stable")
    eA = order[:, 0].astype(np.int64)
    eB = order[:, 1].astype(np.int64)

    plans = []
    maxc = np.zeros(NUM_EXPERTS, dtype=np.int64)
    for i in range(N_CORES):
        lo = i * NSH
        a = eA[lo:lo + NSH]
        b = eB[lo:lo + NSH]
        cnts = np.bincount(np.concatenate([a, b]), minlength=NUM_EXPERTS)
        maxc = np.maximum(maxc, cnts)
        plans.append((a, b))
    cpes = tuple(int(v) for v in np.ceil(maxc / 512.0).astype(np.int64))
    return plans, cpes


def _prep_sparse(x, C, gate_type_embed, Wr, br, W1, W2, plans, cpes):
    import ml_dtypes
    bf = ml_dtypes.bfloat16
    x = np.ascontiguousarray(np.asarray(x, dtype=np.float32))
    G = np.asarray(gate_type_embed, dtype=np.float32)
    Wr = np.asarray(Wr, dtype=np.float32)
    br = np.asarray(br, dtype=np.float32)
    W1 = np.asarray(W1, dtype=np.float32)
    W2 = np.asarray(W2, dtype=np.float32)

    gg = np.zeros((NUM_GATE_TYPES + 1, NUM_EXPERTS + 1), dtype=np.float32)
    gg[0:NUM_GATE_TYPES, 0:NUM_EXPERTS] = G
    gg[0:NUM_GATE_TYPES, NUM_EXPERTS] = 1.0

    w1s = W1.transpose(1, 0, 2).reshape(P, 8 * 256).astype(bf)
    w2s = W2.reshape(8, 2, P, H).transpose(2, 0, 1, 3).reshape(P, 2048).astype(bf)

    caps = [int(c) * 512 for c in cpes]
    bases = np.concatenate([[0], np.cumsum(caps)]).astype(np.int64)
    S = int(bases[-1])
    in_maps = []
    for i in range(N_CORES):
        lo, hi = i * NSH, (i + 1) * NSH
        xs = x[lo:hi]
        xT = np.zeros((P, NS), dtype=np.float32)
        xT[:, :NSH] = xs.T
        cs = C[lo:hi]
        cta = np.zeros((NUM_GATE_TYPES + 1, NS), dtype=np.float32)
        cta[0:NUM_GATE_TYPES, :NSH] = cs.T
        cta[NUM_GATE_TYPES, :] = 1.0

        a, b = plans[i]
        # slot assignment: per expert, rankA nodes then rankB nodes
        slot_node = np.zeros(S, dtype=np.int64)       # node per slot (pad=0)
        idxA = np.zeros(NS, dtype=np.int32)
        idxB = np.zeros(NS, dtype=np.int32)
        for e in range(NUM_EXPERTS):
            na = np.where(a == e)[0]
            nb = np.where(b == e)[0]
            base = int(bases[e])
            ca, cb = len(na), len(nb)
            slot_node[base:base + ca] = na
            slot_node[base + ca:base + ca + cb] = nb
            idxA[na] = base + np.arange(ca, dtype=np.int32)
            idxB[nb] = base + ca + np.arange(cb, dtype=np.int32)

        xg = np.ascontiguousarray(xs.T[:, slot_node].astype(bf))
        # node-grid [P, NG] layout: node n -> (p=n%128, g=n//128)
        idxA_g = idxA.reshape(NG, P).T.copy()
        idxB_g = idxB.reshape(NG, P).T.copy()
        mA = np.zeros((NS, NUM_EXPERTS), dtype=np.float32)
        mB = np.zeros((NS, NUM_EXPERTS), dtype=np.float32)
        mA[np.arange(NSH), a] = 1.0
        mB[np.arange(NSH), b] = 1.0
        mA_g = np.ascontiguousarray(
            mA.reshape(NG, P, NUM_EXPERTS).transpose(1, 0, 2))
        mB_g = np.ascontiguousarray(
            mB.reshape(NG, P, NUM_EXPERTS).transpose(1, 0, 2))

        in_maps.append({
            "xT": np.ascontiguousarray(xT),
            "xg": xg,
            "cta": np.ascontiguousarray(cta),
            "wg": np.ascontiguousarray(Wr),
            "gg": gg,
            "w1s": np.ascontiguousarray(w1s),
            "w2s": np.ascontiguousarray(w2s),
            "mA": mA_g, "mB": mB_g,
            "idxA": np.ascontiguousarray(idxA_g),
            "idxB": np.ascontiguousarray(idxB_g),
        })
    return in_maps


def kernel(x, edge_gate_type, edge_index, gate_type_embed, Wr, br,
           W1, b1, W2, b2, ln_gamma, ln_beta):
    b1a = np.asarray(b1); b2a = np.asarray(b2)
    ga = np.asarray(ln_gamma); ba = np.asarray(ln_beta)
    if np.any(b1a) or np.any(b2a) or np.any(ba) or not np.allclose(ga, 1.0):
        return _fallback_numpy(x, edge_gate_type, edge_index, gate_type_embed,
                               Wr, br, W1, b1, W2, b2, ln_gamma, ln_beta)

    from concourse.bass_utils import run_bass_kernel_spmd

    x = np.ascontiguousarray(np.asarray(x, dtype=np.float32))
    dst = np.asarray(edge_index)[1].astype(np.int64)
    egt = np.asarray(edge_gate_type).astype(np.int64)
    C = np.bincount(dst * NUM_GATE_TYPES + egt,
                    minlength=N * NUM_GATE_TYPES).reshape(
                        N, NUM_GATE_TYPES).astype(np.float32)

    include_br = bool(np.any(np.asarray(br)))
    if not include_br:
        # sparse top-2 path (compute only selected experts)
        plans, cpe = _plan(x, C, gate_type_embed, Wr, br)
        key = ("sparse", cpe)
        if key not in _PROGRAM_CACHE:
            _PROGRAM_CACHE[key] = _build_sparse(cpe)
        nc = _PROGRAM_CACHE[key]
        in_maps = _prep_sparse(x, C, gate_type_embed, Wr, br, W1, W2,
                               plans, cpe)
    else:
        key = ("dense", include_br)
        if key not in _PROGRAM_CACHE:
            _PROGRAM_CACHE[key] = _build_program(include_br)
        nc = _PROGRAM_CACHE[key]
        in_maps = _prep_inputs(x, C, gate_type_embed, Wr, br, W1, W2)

    res = run_bass_kernel_spmd(nc, in_maps, core_ids=list(range(N_CORES)))
    out = np.concatenate([res.results[i]["out"] for i in range(N_CORES)],
                         axis=0)
    return _patch_ambiguous(
        out, x, C, np.asarray(gate_type_embed, dtype=np.float32),
        np.asarray(Wr, dtype=np.float32), np.asarray(br, dtype=np.float32),
        np.asarray(W1, dtype=np.float32), np.asarray(b1, dtype=np.float32),
        np.asarray(W2, dtype=np.float32), np.asarray(b2, dtype=np.float32),
        np.asarray(ln_gamma, dtype=np.float32),
        np.asarray(ln_beta, dtype=np.float32))



# revision 2
# speedup vs baseline: 9.0264x; 9.0264x over previous
"""GateTypeExpertLayer kernel for 8 Trainium2 NeuronCores (SPMD).

v4: instruction-count-minimized design. Through this execution stack every
device instruction costs ~25-70us (measured: DVE ~26us, matmul ~69us,
DMA ~52us), so the kernel is built to minimize instruction count:

  - Host computes routing exactly (histogram -> router logits -> top-2 ->
    softmax gates) and sorts nodes by their unordered expert *pair* so each
    contiguous slot-run needs exactly 2 experts. Host also un-permutes the
    output. (The previous baseline already hosted the histogram + routing
    plan; this moves the rest of the routing bookkeeping there too.)
  - Device: per strip (<=512 slots, one expert pair): 4 W1 matmuls ->
    one batched Gelu -> one batched scale by per-slot gate weights
    (broadcast via a step-0-partition DMA load) -> 4 W2 matmuls that
    accumulate BOTH experts into one PSUM tile (the top-2 combine is free,
    done by PSUM accumulation on pre-scaled activations) -> one copy into
    a resident [128, S] bf16 accumulator.
  - One dma_start_transpose converts feat-major [128, S] to node-partition
    [128, S/128, 128]; LayerNorm runs as ~8 whole-tensor instructions; one
    contiguous DMA writes the output.

Per core: ~450 instructions total (vs ~2900 in the previous version).
"""

import numpy as np
import sys

sys.path.insert(0, "/opt/trn_rl_repo")

N_CORES = 8
N = 100000
H = 128
NUM_EXPERTS = 8
NUM_GATE_TYPES = 20
LN_EPS = 1e-5
NSH = N // N_CORES            # 12500 nodes per core
P = 128
STRIP = 512                   # max matmul free dim / PSUM bank
MAX_S = 18944                 # SBUF budget cap on padded slots per core

_PROGRAM_CACHE = {}


def _histogram(edge_index, edge_gate_type):
    dst = np.asarray(edge_index)[1].astype(np.int64)
    egt = np.asarray(edge_gate_type).astype(np.int64)
    return np.bincount(dst * NUM_GATE_TYPES + egt,
                       minlength=N * NUM_GATE_TYPES).reshape(
                           N, NUM_GATE_TYPES).astype(np.float32)


def _route(x, C, gate_type_embed, Wr, br):
    """Replicate the reference router in fp32 on host.

    Returns eA, eB (top-2 expert ids) and wA, wB (softmax gates)."""
    x = np.asarray(x, dtype=np.float32)
    G = np.asarray(gate_type_embed, dtype=np.float32)
    cnt = C.sum(axis=1, dtype=np.float32)
    gate = np.where(cnt[:, None] > 0,
                    (C @ G) / np.maximum(cnt, 1.0)[:, None],
                    0.0).astype(np.float32)
    logits = (x @ np.asarray(Wr, np.float32)
              + np.asarray(br, np.float32)[None, :] + gate)
    order = np.argsort(-logits, axis=1, kind="stable")
    eA = order[:, 0]
    eB = order[:, 1]
    lA = np.take_along_axis(logits, eA[:, None], 1)[:, 0].astype(np.float64)
    lB = np.take_along_axis(logits, eB[:, None], 1)[:, 0].astype(np.float64)
    wA = (1.0 / (1.0 + np.exp(lB - lA))).astype(np.float32)
    wB = (1.0 - wA).astype(np.float32)
    return eA, eB, wA, wB


def _plan(eA, eB):
    """Pair-sort plan shared by all cores (SPMD: one program).

    Returns (strips, S, per_core) where strips is a tuple of
    (offset, n, expert_a, expert_b) compile-time constants and per_core[i]
    holds (slot_node, slot_wsel, valid) indexing arrays."""
    u = np.minimum(eA, eB)
    v = np.maximum(eA, eB)
    key = (u * NUM_EXPERTS + v).astype(np.int64)

    counts = np.zeros((N_CORES, NUM_EXPERTS * NUM_EXPERTS), np.int64)
    for i in range(N_CORES):
        k = key[i * NSH:(i + 1) * NSH]
        counts[i] = np.bincount(k, minlength=NUM_EXPERTS * NUM_EXPERTS)
    cap = ((counts.max(axis=0) + P - 1) // P) * P     # 128-aligned, shared
    active = np.where(cap > 0)[0]

    strips = []
    seg_base = {}
    off = 0
    for kk in active:
        seg_base[int(kk)] = off
        a, b = int(kk) // NUM_EXPERTS, int(kk) % NUM_EXPERTS
        rem = int(cap[kk])
        o = off
        while rem > 0:
            n = min(STRIP, rem)
            strips.append((o, n, a, b))
            o += n
            rem -= n
        off += int(cap[kk])
    S = off
    assert S % P == 0

    per_core = []
    for i in range(N_CORES):
        lo = i * NSH
        k = key[lo:lo + NSH]
        slot_node = np.zeros(S, np.int64)
        valid = np.zeros(S, bool)
        is_A_first = np.zeros(S, bool)   # whether pair-min expert == eA
        for kk in active:
            nodes = np.where(k == kk)[0]
            base = seg_base[int(kk)]
            c = len(nodes)
            slot_node[base:base + c] = nodes + lo
            valid[base:base + c] = True
            a = int(kk) // NUM_EXPERTS
            is_A_first[base:base + c] = (eA[nodes + lo] == a)
        per_core.append((slot_node, valid, is_A_first))
    return tuple(strips), S, per_core


def _build_v4(strips, S, reps=1):
    import concourse.bacc as bacc
    import concourse.tile as tile
    import concourse.mybir as mybir
    import concourse.bass as bass

    f32 = mybir.dt.float32
    bf16 = mybir.dt.bfloat16
    AF = mybir.ActivationFunctionType
    OP = mybir.AluOpType
    G = S // P

    nc = bacc.Bacc("TRN2", target_bir_lowering=False, debug=False,
                   num_devices=N_CORES)

    xg = nc.dram_tensor("xg", [P, S], bf16, kind="ExternalInput").ap()
    wgd = nc.dram_tensor("wgd", [2, S], bf16, kind="ExternalInput").ap()
    w1s = nc.dram_tensor("w1s", [P, 2048], bf16, kind="ExternalInput").ap()
    w2s = nc.dram_tensor("w2s", [P, 2048], bf16, kind="ExternalInput").ap()
    outd = nc.dram_tensor("outd", [P, G, H], bf16, kind="ExternalOutput").ap()

    def pbc(sl, count):
        # DRAM partition-broadcast: read one row into all partitions
        ap = [list(d) for d in sl.ap]
        return bass.AP(tensor=sl.tensor, offset=sl.offset,
                       ap=[[0, count]] + ap[1:])

    def bc(sl, count):
        ap = [list(d) for d in sl.ap]
        return bass.AP(tensor=sl.tensor, offset=sl.offset,
                       ap=ap + [[0, count]])

    def wexp(sl):
        # [P, 2, n] -> [P, 2, 2, n] with a step-0 dim for the hidden halves
        ap = [list(d) for d in sl.ap]
        return bass.AP(tensor=sl.tensor, offset=sl.offset,
                       ap=[ap[0], ap[1], [0, 2], ap[2]])

    with tile.TileContext(nc) as tc:
        with tc.tile_pool(name="const", bufs=1) as constp:
            w1_sb = constp.tile([P, 2048], bf16)
            nc.sync.dma_start(out=w1_sb[:], in_=w1s[:])
            w2_sb = constp.tile([P, 2048], bf16)
            nc.sync.dma_start(out=w2_sb[:], in_=w2s[:])
            eps_sb = constp.tile([P, 1], f32)
            nc.vector.memset(eps_sb[:], LN_EPS)
            xg_sb = constp.tile([P, S], bf16)
            nc.sync.dma_start(out=xg_sb[:], in_=xg[:])
            # per-slot gate weights broadcast to all 128 partitions
            wball = constp.tile([P, 2, S], bf16)
            nc.sync.dma_start(out=wball[:, 0, :], in_=pbc(wgd[0:1, :], P))
            nc.sync.dma_start(out=wball[:, 1, :], in_=pbc(wgd[1:2, :], P))

            def _body():
                with tc.tile_pool(name="work", bufs=1) as wp, \
                     tc.tile_pool(name="hsp", bufs=2) as hsp, \
                     tc.tile_pool(name="hpsum", bufs=1, space="PSUM") as hps, \
                     tc.tile_pool(name="ypsum", bufs=2, space="PSUM") as yps:
                    yAll = wp.tile([P, S], bf16, tag="big")
                    for (off, n, a, b) in strips:
                        hp = hps.tile([P, 2, 2, STRIP], f32, tag="hp")
                        for ei, e in enumerate((a, b)):
                            for m in range(2):
                                nc.tensor.matmul(
                                    out=hp[:, ei, m, 0:n],
                                    lhsT=w1_sb[:, e * 256 + m * P:
                                               e * 256 + (m + 1) * P],
                                    rhs=xg_sb[:, off:off + n],
                                    start=True, stop=True)
                        hs = hsp.tile([P, 2, 2, STRIP], bf16, tag="hs")
                        nc.scalar.activation(out=hs[:, :, :, 0:n],
                                             in_=hp[:, :, :, 0:n],
                                             func=AF.Gelu)
                        nc.vector.tensor_tensor(
                            out=hs[:, :, :, 0:n], in0=hs[:, :, :, 0:n],
                            in1=wexp(wball[:, :, off:off + n]), op=OP.mult)
                        yT = yps.tile([P, STRIP], f32, tag="yT")
                        k = 0
                        for ei, e in enumerate((a, b)):
                            for m in range(2):
                                nc.tensor.matmul(
                                    out=yT[:, 0:n],
                                    lhsT=w2_sb[:, (2 * e + m) * P:
                                               (2 * e + m + 1) * P],
                                    rhs=hs[:, ei, m, 0:n],
                                    start=(k == 0), stop=(k == 3))
                                k += 1
                        nc.vector.tensor_copy(out=yAll[:, off:off + n],
                                              in_=yT[:, 0:n])

                    # ---- LayerNorm over all nodes, then store ----
                    yn = wp.tile([P, G, H], bf16, tag="yn")
                    nc.sync.dma_start_transpose(yn[:], yAll[:])
                    mu = wp.tile([P, G], f32, tag="mu")
                    nc.vector.tensor_reduce(out=mu[:], in_=yn[:],
                                            axis=mybir.AxisListType.X,
                                            op=OP.add)
                    nc.vector.tensor_scalar_mul(mu[:], mu[:], 1.0 / H)
                    nc.vector.tensor_tensor(out=yn[:], in0=yn[:],
                                            in1=bc(mu[:], H), op=OP.subtract)
                    sq = wp.tile([P, G, H], bf16, tag="big")
                    nc.scalar.activation(out=sq[:], in_=yn[:], func=AF.Square)
                    vr = wp.tile([P, G], f32, tag="vr")
                    nc.vector.tensor_reduce(out=vr[:], in_=sq[:],
                                            axis=mybir.AxisListType.X,
                                            op=OP.add)
                    sd = wp.tile([P, G], f32, tag="sd")
                    nc.scalar.activation(out=sd[:], in_=vr[:], func=AF.Sqrt,
                                         bias=eps_sb[:], scale=1.0 / H)
                    nc.vector.reciprocal(sd[:], sd[:])
                    nc.vector.tensor_tensor(out=yn[:], in0=yn[:],
                                            in1=bc(sd[:], H), op=OP.mult)
                    nc.sync.dma_start(out=outd[:], in_=yn[:])

            for _rep in range(reps):
                _body()

    nc.compile()
    return nc


def _prep(x, eA, eB, wA, wB, W1, W2, strips, S, per_core):
    import ml_dtypes
    bf = ml_dtypes.bfloat16
    x = np.asarray(x, dtype=np.float32)
    W1 = np.asarray(W1, dtype=np.float32)
    W2 = np.asarray(W2, dtype=np.float32)

    w1s = W1.transpose(1, 0, 2).reshape(P, NUM_EXPERTS * 256).astype(bf)
    w2s = W2.reshape(NUM_EXPERTS, 2, P, H).transpose(2, 0, 1, 3).reshape(
        P, NUM_EXPERTS * 256).astype(bf)

    in_maps = []
    for i in range(N_CORES):
        slot_node, valid, is_A_first = per_core[i]
        xg = np.zeros((P, S), dtype=bf)
        xg[:, valid] = x[slot_node[valid]].T.astype(bf)
        # row 0: weight of pair-min expert; row 1: weight of pair-max expert
        wgd = np.zeros((2, S), dtype=np.float32)
        wa = wA[slot_node[valid]]
        wb = wB[slot_node[valid]]
        first = is_A_first[valid]
        wgd[0, valid] = np.where(first, wa, wb)
        wgd[1, valid] = np.where(first, wb, wa)
        in_maps.append({
            "xg": np.ascontiguousarray(xg),
            "wgd": np.ascontiguousarray(wgd.astype(bf)),
            "w1s": np.ascontiguousarray(w1s),
            "w2s": np.ascontiguousarray(w2s),
        })
    return in_maps


def _fallback_numpy(x, edge_gate_type, edge_index, gate_type_embed, Wr, br,
                    W1, b1, W2, b2, ln_gamma, ln_beta):
    # exact reference recomputation on host (only for unexpected inputs)
    import math
    x = np.asarray(x, dtype=np.float32)
    n = x.shape[0]
    C = _histogram(edge_index, edge_gate_type)
    G = np.asarray(gate_type_embed, dtype=np.float32)
    cnt = C.sum(axis=1, dtype=np.float32)
    gate = np.where(cnt[:, None] > 0,
                    (C @ G) / np.maximum(cnt, 1.0)[:, None], 0.0)
    rl = x @ np.asarray(Wr, np.float32) + np.asarray(br, np.float32) + gate
    order = np.argsort(-rl, axis=1, kind="stable")
    tki = order[:, :2]
    tkl = np.take_along_axis(rl, tki, 1)
    m = tkl.max(axis=1, keepdims=True)
    e = np.exp(tkl - m)
    tkg = e / e.sum(axis=1, keepdims=True)
    W1 = np.asarray(W1, np.float32)
    b1 = np.asarray(b1, np.float32)
    W2 = np.asarray(W2, np.float32)
    b2 = np.asarray(b2, np.float32)
    out = np.zeros((n, H), np.float32)
    from scipy.special import erf  # noqa: F401  (fallback only)
    for kk in range(2):
        ei = tki[:, kk]
        g = tkg[:, kk]
        for ex in range(NUM_EXPERTS):
            sel = np.where(ei == ex)[0]
            if len(sel) == 0:
                continue
            z = x[sel] @ W1[ex] + b1[ex]
            h = 0.5 * z * (1.0 + erf(z / np.sqrt(2.0)))
            out[sel] += g[sel, None] * (h @ W2[ex] + b2[ex])
    mu = out.mean(axis=1, keepdims=True)
    var = ((out - mu) ** 2).mean(axis=1, keepdims=True)
    o = (out - mu) / np.sqrt(var + LN_EPS)
    return (o * np.asarray(ln_gamma, np.float32)
            + np.asarray(ln_beta, np.float32)).astype(np.float32)


def kernel(x, edge_gate_type, edge_index, gate_type_embed, Wr, br,
           W1, b1, W2, b2, ln_gamma, ln_beta):
    b1a = np.asarray(b1); b2a = np.asarray(b2)
    ga = np.asarray(ln_gamma); ba = np.asarray(ln_beta)
    if np.any(b1a) or np.any(b2a) or np.any(ba) or not np.allclose(ga, 1.0):
        return _fallback_numpy(x, edge_gate_type, edge_index, gate_type_embed,
                               Wr, br, W1, b1, W2, b2, ln_gamma, ln_beta)

    x = np.ascontiguousarray(np.asarray(x, dtype=np.float32))
    C = _histogram(edge_index, edge_gate_type)
    eA, eB, wA, wB = _route(x, C, gate_type_embed, Wr, br)
    strips, S, per_core = _plan(eA, eB)
    if S > MAX_S:
        return _fallback_numpy(x, edge_gate_type, edge_index, gate_type_embed,
                               Wr, br, W1, b1, W2, b2, ln_gamma, ln_beta)

    from concourse.bass_utils import run_bass_kernel_spmd

    key = ("v4", strips, S)
    if key not in _PROGRAM_CACHE:
        _PROGRAM_CACHE[key] = _build_v4(strips, S)
    nc = _PROGRAM_CACHE[key]
    in_maps = _prep(x, eA, eB, wA, wB, W1, W2, strips, S, per_core)
    res = run_bass_kernel_spmd(nc, in_maps, core_ids=list(range(N_CORES)))

    out = np.empty((N, H), dtype=np.float32)
    for i in range(N_CORES):
        o = np.asarray(res.results[i]["outd"], dtype=np.float32)
        y_slots = o.transpose(1, 0, 2).reshape(S, H)
        slot_node, valid, _ = per_core[i]
        out[slot_node[valid]] = y_slots[valid]
    return out
